# revision 62
# baseline (speedup 1.0000x reference)
"""Trainium2 Bass kernel for the DVS-SNN problem.

Model (per reference):
    for t in 0..T-1:
        i1 = x_t @ w1.T + spk @ w_rec.T
        v1 = v1 + i1 - LEAK ; spk = (v1 >= 1) ; v1 -= spk
        v2 = max(v2 + spk @ w2.T - OUTPUT_LEAK, 0) ; out_sum += v2
    return out_sum / T

Strategy: data-parallel over batch (64 = 8 cores x 8). Per core, one
interleaved program:
  Warm-up: ~8us of dummy 512-col matmuls release the PE HAM clock-gate
      (default 1.2 GHz) during the initial DMA wait.
  Phase A (time-parallel): nxp[h, 32t+8j+b] = -(S*(xp - LEAK)).
      Main pass: fp16 xh @ (-S*w1h). Correction pass: TWO regular fp8
      matmuls per (k,j) at HALF width -- the tiny cross terms
      (w1h*xl + w1l*xh) are pair-summed over adjacent timesteps on the
      host and applied at even steps (sim rel err 1.19e-2 vs 2e-2 gate).
      Regular fp8 keeps FWL (~25ns hidden LDW); DoubleRow's no-FWL
      LDWEIGHTS would cost a serial ~213ns per (k,j), ~80us of PE.
      Combine on ScalarE (scale) + DVE bias-add / even-add into nxp.
      x streams in 4 windows, double-buffered (xpool bufs=2) on the
      sync/gpsimd queues only -- dma_starts block their issuing queue,
      and sharing with the combine COPYs stalls the strict-FIFO DVE.
  Phase B (sequential scan): state nG = S - U(t-1) - X(t), tiles
      [128, 32]. w_rec fp16 (scaled S=64) stationaries, fp8 spike moving
      (mixed-dtype matmul). Critical path per step (~1us): hop +
      ONE plain tt DVE op spk = (psB >= nG) + hop + the 16 LDW+MM burst
      (25ns NX-floor cadence). Three DVE update ops run in the burst's
      shadow; phase A matmuls fill the hop/spike window on the PE.
  Phase C (per column tile, interleaved): i2 = spk @ w2.T with fp16 w2
      stationary x fp8 spikes; (i2-OL)/T on ScalarE; carry-seeded
      incremental relu-scan + reduce per tile (sentinel resets batch
      lanes), so the tail is just a DMA of osum.
"""

import os
import numpy as np

B, T, C, H, O = 64, 256, 2048, 512, 11
THRESHOLD = 1.0
LEAK = 0.003
OUTPUT_LEAK = LEAK * 0.5

NCORES = 8
BL = B // NCORES          # batch per core = 8
BT = T * BL               # 2048 moving columns per core
P = 128
KC = C // P               # 16 contraction chunks for phase A
KH = H // P               # 4 H chunks
NTILE = 512               # max phase A psum tile (64 timesteps x 8 batch)
XWINS = [(0, 16), (16, 40), (40, 112), (112, 196), (196, 256)]  # x windows
XWMAX = 84                # max window width, timesteps

SCALE = 64.0              # v1 dynamics scale (wrec fp16 subnormal safety)
# fp8 correction-factor scalings (all powers of two; products land at
# -S * 2^CSH * (w1h*xl + w1l*xh)).
SH_W, SH_WL, SH_X, SH_XL = 5, 16, 3, 14
CSH = float(2 ** (SH_W + SH_XL))  # = 2^(SH_WL+SH_X) = 2^19


def _sched(t_steps):
    """Column tiles: ramp from a SMALL first tile (the scan starts once
    tile 0's inputs land -- less startup DMA) to wide tiles. Widths must
    ASCEND (phase C's zero-delta trailing-column trick relies on it) and
    consecutive widths must stay close or the PE queue between bursts
    overloads with the next tile's phase-A matmuls."""
    assert t_steps == 256
    bounds = [0, 16, 40, 72, 112, 152, 196, 256]
    return list(zip(bounds[:-1], bounds[1:]))


def build_nc(t_steps=T):
    """Build the Bass program (same program for all 8 cores)."""
    from contextlib import ExitStack

    import concourse.tile as tile
    from concourse import bacc, mybir

    f32 = mybir.dt.float32
    f16 = mybir.dt.float16
    f8 = mybir.dt.float8e4
    alu = mybir.AluOpType
    ACT = mybir.ActivationFunctionType
    DR = mybir.MatmulPerfMode.DoubleRow

    nc = bacc.Bacc("TRN2", target_bir_lowering=False, debug=False,
                   num_devices=NCORES)

    # ---- DRAM I/O ----
    xh_d = nc.dram_tensor("xh", [C, BT], f16, kind="ExternalInput")
    xdr_d = nc.dram_tensor("xdr", [C, 2, BT // 2], f8, kind="ExternalInput")
    w1m_d = nc.dram_tensor("w1m", [C, H], f16, kind="ExternalInput")
    wdr_d = nc.dram_tensor("wdr", [C, 2, H], f8, kind="ExternalInput")
    wrt_d = nc.dram_tensor("wrt", [H, H], f16, kind="ExternalInput")
    w2t_d = nc.dram_tensor("w2t", [H, O], f16, kind="ExternalInput")
    out_d = nc.dram_tensor("out", [O, BL], f32, kind="ExternalOutput")

    TH_S = SCALE * THRESHOLD
    sched = _sched(t_steps)

    with tile.TileContext(nc) as tc, ExitStack() as ctx:
        perm = ctx.enter_context(tc.tile_pool(name="perm", bufs=1))

        def ptile(shape, dt_, tag):
            return perm.tile(shape, dt_, tag=tag, name=tag)

        nxp = ptile([P, 32 * t_steps], f32, "nxp")          # -(S*(xp-LEAK))
        spk8 = ptile([P, 32 * t_steps], f8, "spk8")         # {0,1} spikes
        nU = ptile([P, 32], f32, "nU")                      # scan state
        # phase C runs INCREMENTALLY per column tile: per batch lane the
        # tile's deltas are scanned with a relu-scan seeded by a carry
        # column (previous tile's final v2); a -inf sentinel column ahead
        # of the carry resets the running value at batch boundaries.
        WMAX = max(b - a for a, b in _sched(t_steps))
        d_c = ptile([O, BL * (WMAX + 2)], f32, "d_c")       # [o,(b,2+w)]
        v2_c = ptile([O, BL * (WMAX + 2)], f32, "v2_c")     # scan output
        zer_c = ptile([O, BL * (WMAX + 2)], f32, "zer_c")
        osum = ptile([O, BL], f32, "osum")
        part = ptile([O, BL], f32, "part")
        if os.environ.get("SNN_FLATC", "0") == "1":
            d_flat = ptile([O, BL * (t_steps + 1)], f32, "d_flat")
            v2f = ptile([O, BL * (t_steps + 1)], f32, "v2f")
            zerf = ptile([O, BL * (t_steps + 1)], f32, "zerf")

        w1m_sb, wdr_sb = [], []
        wr_sb, w2_sb = [], []
        xwin = {}                                            # (k) -> tiles

        # bufs=2: window w+1's DMA streams into the spare slot while
        # window w is still being read -- otherwise the DMA waits on the
        # last reader and the scheduler head-of-line-blocks the PE on an
        # x-gated matmul for ~5us (which also re-throttles the PE clock).
        xpool = ctx.enter_context(tc.tile_pool(name="xp", bufs=2))
        tmp_pool = ctx.enter_context(tc.tile_pool(name="tmpA", bufs=2))
        psA_pool = ctx.enter_context(tc.tile_pool(name="psA", bufs=1,
                                                  space="PSUM"))
        psB_pool = ctx.enter_context(tc.tile_pool(name="psB", bufs=2,
                                                  space="PSUM"))
        psV_pool = ctx.enter_context(tc.tile_pool(name="psV", bufs=2,
                                                  space="PSUM"))

        # Mid-scan x-window DMAs go ONLY on sync/gpsimd: a dma_start
        # blocks its issuing queue on HW-queue/sem waits for ~us; with
        # combine COPYs sharing the scalar queue that stalled the DVE
        # FIFO (combine-ADD between scan state ops) for ~10us per window.
        dma_engines = [nc.sync, nc.gpsimd]
        # Startup-only DMAs may also use scalar (idle pre-scan).
        dma_engines_w = [nc.sync, nc.gpsimd, nc.scalar]

        # ---------- PE HAM warm-up ----------
        # The PE clock-gate defaults to 4/8 (1.2 GHz) and only releases
        # after ~3.4us of sustained PE activity. Without this, the whole
        # phase A ramp for tile 0 runs at half clock while waiting out
        # the DMA stream (~30us wasted). Burn ~6us of dummy LDW+MM pairs
        # during the initial DMA wait (PE would idle anyway).
        # 512-col matmuls keep the PE array continuously busy (tiny MMs
        # have ~50% duty at the NX dispatch floor and never trip the
        # HAM busy-window detector).
        warm = ptile([P, NTILE], f16, "warm")
        nc.vector.memset(warm[:], 0.0)
        warm_ps = psA_pool.tile([P, NTILE], f32, tag="psA0", name="psA",
                                padded_shape=[P, NTILE])
        for _ in range(20):
            nc.tensor.matmul(warm_ps[:], warm[:, 0:P], warm[:],
                             start=True, stop=True)

        def dma_xwin(w, k, eng=None, only=None):
            """Stream x window w for contraction chunk k on a HWDGE queue.
            xdr carries timestep-PAIR-summed correction factors, so its
            column count is half the window's. only='xh'/'xd' issues a
            single tensor (startup orders main-pass inputs first)."""
            w0, w1 = XWINS[w]
            wl = (w1 - w0) * BL
            eng = eng if eng is not None else dma_engines[k % 2]
            csl = slice(k * P, (k + 1) * P)
            if only in (None, 'xh'):
                xh_t = xpool.tile([P, wl], f16, tag=f"xh_{k}",
                                  name=f"xh_{k}",
                                  padded_shape=[P, XWMAX * BL])
                eng.dma_start(out=xh_t[:],
                              in_=xh_d.ap()[csl, w0 * BL:w1 * BL])
                xwin[(w, k, 'h')] = xh_t
            if only in (None, 'xd'):
                xd_t = xpool.tile([P, 2, wl // 2], f8, tag=f"xd_{k}",
                                  name=f"xd_{k}",
                                  padded_shape=[P, 2, XWMAX * BL // 2])
                eng.dma_start(
                    out=xd_t[:],
                    in_=xdr_d.ap()[csl, :, w0 * BL // 2:w1 * BL // 2])
                xwin[(w, k, 'd')] = xd_t

        # ---------- Phase A work for one column tile, as a work list ----------
        # Correction pass: TWO regular fp8 matmuls at HALF width (pair-
        # summed over adjacent timesteps). Regular matmuls keep FWL
        # (~25ns hidden LDW); DoubleRow's 256-col no-FWL LDWEIGHTS costs
        # a serial ~213ns per (k,j) -- ~80us of PE across the scan.
        def a_tile_work(s0, s1):
            ntile = (s1 - s0) * BL
            w = next(i for i, (a, b) in enumerate(XWINS) if a <= s0 < b)
            base = s0 - XWINS[w][0]
            psA = [None] * 2
            psC = [None] * 2
            items = []
            for jp in range(2):
                def alloc(jp=jp):
                    for jj in range(2):
                        psA[jj] = psA_pool.tile([P, ntile], f32,
                                                tag=f"psA{jj}", name="psA",
                                                padded_shape=[P, NTILE])
                        psC[jj] = psA_pool.tile([P, ntile // 2], f32,
                                                tag=f"psC{jj}", name="psC",
                                                padded_shape=[P, NTILE // 2])
                items.append(alloc)
                # main pass first (its inputs stream first at startup),
                # correction pass after
                for k in range(KC):
                    for jj in range(2):
                        def mm_main(k=k, jp=jp, jj=jj):
                            xh_t = xwin[(w, k, 'h')]
                            xsl = slice(base * BL, base * BL + ntile)
                            j = 2 * jp + jj
                            jsl = slice(j * P, (j + 1) * P)
                            nc.tensor.matmul(
                                psA[jj][:], w1m_sb[k][:][:, jsl],
                                xh_t[:, xsl],
                                start=(k == 0), stop=(k == KC - 1))
                        items.append(mm_main)
                for k in range(KC):
                    for jj in range(2):
                        def mm_corr(k=k, jp=jp, jj=jj):
                            xd_t = xwin[(w, k, 'd')]
                            csl = slice(base * BL // 2,
                                        (base * BL + ntile) // 2)
                            j = 2 * jp + jj
                            jsl = slice(j * P, (j + 1) * P)
                            nc.tensor.matmul(
                                psC[jj][:], wdr_sb[k][:][:, 0, jsl],
                                xd_t[:, 0, csl],
                                start=(k == 0), stop=False)
                            nc.tensor.matmul(
                                psC[jj][:], wdr_sb[k][:][:, 1, jsl],
                                xd_t[:, 1, csl],
                                start=False, stop=(k == KC - 1))
                        items.append(mm_corr)
                for jj in range(2):
                    def combine(jj=jj, jp=jp):
                        j = 2 * jp + jj
                        tmp = tmp_pool.tile([P, ntile // 2], f32, tag="tmpA",
                                            name="tmpA",
                                            padded_shape=[P, NTILE // 2])
                        nc.scalar.activation(tmp[:], psC[jj][:], ACT.Copy,
                                             bias=0.0, scale=1.0 / CSH)
                        # nxp = psA + S*LEAK everywhere; pair-summed corr
                        # lands on EVEN timesteps only. ODD columns are
                        # written by the (idle) Scalar engine straight
                        # from PSUM; EVEN columns need two tensors so one
                        # DVE stt does (tmp + S*LEAK) + psA. Chunked so
                        # the strict-FIFO DVE never blocks the spike long.
                        dest = nxp[:].rearrange(
                            "p (t2 two j b) -> p t2 two j b", two=2,
                            j=KH, b=BL)
                        pA = psA[jj][:].rearrange("p (t2 two b) -> p t2 two b",
                                                  two=2, b=BL)
                        tA = tmp[:].rearrange("p (t2 b) -> p t2 b", b=BL)
                        h0 = s0 // 2
                        nst2 = (s1 - s0) // 2
                        nc.scalar.activation(
                            dest[:, h0:h0 + nst2, 1, j, :],
                            pA[:, :, 1, :], ACT.Copy,
                            bias=SCALE * LEAK, scale=1.0)
                        for q0 in range(0, nst2, 8):
                            q1 = min(q0 + 8, nst2)
                            nc.vector.scalar_tensor_tensor(
                                dest[:, h0 + q0:h0 + q1, 0, j, :],
                                tA[:, q0:q1, :], SCALE * LEAK,
                                pA[:, q0:q1, 0, :], alu.add, alu.add)
                    items.append(combine)
            return items

        # ---------- Phase C work for one column tile (8 spread items) ------
        # Per tile: 4 matmuls -> psV, scale to deltas in d_c cols 2..2+w,
        # carry-seeded relu-scan, reduce, accumulate into osum. The carry
        # chain serializes tile C-work but each link is ~1.5us of DVE.
        def c_tile_items(n, s0, s1):
            spk_r = spk8[:].rearrange("p (t c b) -> p t c b", c=KH, b=BL)
            hold = {}
            w = s1 - s0
            d_r = d_c[:].rearrange("o (b c) -> o b c", c=WMAX + 2)
            v2_r = v2_c[:].rearrange("o (b c) -> o b c", c=WMAX + 2)

            def mk_mm(k):
                def mm(k=k):
                    if k == 0:
                        hold["psV"] = psV_pool.tile(
                            [O, (s1 - s0) * BL], f32, tag="psV",
                            name="psV", padded_shape=[O, NTILE])
                    nc.tensor.matmul(hold["psV"][:], w2_sb[k][:],
                                     spk_r[:, s0:s1, k, :],
                                     start=(k == 0), stop=(k == KH - 1))
                return mm

            def fin():
                # d[o, b, 2+t] = (i2 - OL)/T, written batch-major
                srcv = hold["psV"][:].rearrange("o (t b) -> o b t", b=BL)
                nc.scalar.activation(d_r[:, :, 2:2 + w], srcv, ACT.Copy,
                                     bias=-OUTPUT_LEAK / float(T),
                                     scale=1.0 / float(T))

            def carry():
                # cols 2+w..2+WMAX stay zero-delta (widths ascend), so the
                # previous tile's carry is always at the LAST scan column.
                if n == 0:
                    nc.vector.memset(d_r[:, :, 1:2], 0.0)
                else:
                    nc.vector.tensor_scalar(
                        d_r[:, :, 1:2],
                        v2_r[:, :, WMAX + 1:WMAX + 2],
                        0.0, None, alu.add)

            def scan():
                # full fixed-width flat 2D scan (op requires 2D operands)
                nc.vector.tensor_tensor_scan(v2_c[:], d_c[:], zer_c[:],
                                             0.0, alu.add, alu.max)

            def red():
                nc.vector.tensor_reduce(out=part[:], in_=v2_r[:, :, 2:2 + w],
                                        axis=mybir.AxisListType.X, op=alu.add)
                nc.vector.tensor_add(osum[:], osum[:], part[:])
            if os.environ.get("SNN_FLATC", "0") == "1":
                def fin_flat():
                    dest = d_flat[:].rearrange(
                        "o (b tp) -> o b tp", tp=t_steps + 1)[:, :, s0:s1]
                    srcv = hold["psV"][:].rearrange("o (t b) -> o b t", b=BL)
                    nc.scalar.activation(dest, srcv, ACT.Copy,
                                         bias=-OUTPUT_LEAK / float(T),
                                         scale=1.0 / float(T))
                return [mk_mm(k) for k in range(KH)] + [fin_flat]
            return [mk_mm(k) for k in range(KH)] + [fin, carry, scan, red]

        # ---------- Weight + first-windows DMAs ----------
        # Round-robin each chunk's four tensors across four engine queues
        # (vector/scalar are idle pre-scan) so the serial ~0.65us
        # per-dma_start issue cost is split 4 ways and chunk k's data
        # lands in chunk order. Window 1 preloads into the spare xpool
        # slot (bufs=2) so no x DMA is due during the early scan.
        # Per-chunk interleave: chunk k's four tensors land together in
        # k order, so tile0's PE consumption tracks the DMA stream with
        # no engine-idle gaps (gaps > 3.4us re-throttle the PE clock).
        # Window 1 is NOT loaded here -- it is issued at scan step 0
        # into the spare xpool slot and streams during the early scan.
        for k in range(KC):
            w1m_k = ptile([P, H], f16, f"w1m_{k}")
            wdr_k = ptile([P, 2, H], f8, f"wdr_{k}")
            dma_engines_w[k % 3].dma_start(
                out=w1m_k[:], in_=w1m_d.ap()[k * P:(k + 1) * P, :])
            dma_engines_w[(k + 1) % 3].dma_start(
                out=wdr_k[:], in_=wdr_d.ap()[k * P:(k + 1) * P, :, :])
            w1m_sb.append(w1m_k)
            wdr_sb.append(wdr_k)
            dma_xwin(0, k, eng=dma_engines_w[(k + 2) % 3])
        for k in range(KH):
            wr_k = ptile([P, H], f16, f"wr_{k}")
            nc.sync.dma_start(out=wr_k[:], in_=wrt_d.ap()[k * P:(k + 1) * P, :])
            wr_sb.append(wr_k)
            w2_k = ptile([P, O], f16, f"w2_{k}")
            nc.sync.dma_start(out=w2_k[:], in_=w2t_d.ap()[k * P:(k + 1) * P, :])
            w2_sb.append(w2_k)
        # phase C init: -inf sentinel column per batch lane (resets the
        # relu-scan running value at batch boundaries); delta columns
        # zeroed once (narrow tiles leave trailing columns zero-delta).
        nc.vector.memset(d_c[:], 0.0)
        nc.vector.memset(
            d_c[:].rearrange("o (b c) -> o b c", c=WMAX + 2)[:, :, 0:1],
            -1e30)
        nc.vector.memset(zer_c[:], 0.0)
        nc.vector.memset(osum[:], 0.0)
        if os.environ.get("SNN_FLATC", "0") == "1":
            nc.vector.memset(zerf[:], 0.0)
            nc.vector.memset(
                d_flat[:].rearrange("o (b tp) -> o b tp",
                                    tp=t_steps + 1)[:, :, t_steps:],
                -1e30)

        # ---------- Build per-step interleave schedule ----------
        # Tile n's A work is emitted across the scan steps of tile n-1
        # (disjoint windows: one tile's PSUM accumulation at a time);
        # tiles 0 and 1 are emitted up front.
        step_work = [[] for _ in range(t_steps + 8)]

        def spread(items, lo, hi):
            span = hi - lo
            for i, it in enumerate(items):
                step_work[lo + (i * span) // len(items)].append(it)

        pre_work = []
        for n, (s0, s1) in enumerate(sched):
            items = a_tile_work(s0, s1)
            if n == 0:
                pre_work.extend(items)
            else:
                # End 2 steps before s0: step t's state update READS
                # nxp[t+1], and step_work items are emitted AFTER the
                # scan ops of their step -- a combine landing at step
                # s0-1 would be emitted after the update that reads it
                # (the framework cannot depend on a future writer).
                spread(items, sched[n - 1][0], s0 - 2)
            for i, cit in enumerate(c_tile_items(n, s0, s1)):
                step_work[s1 + i].append(cit)
        # Later x windows: emit window w's DMAs just BEFORE the first
        # window-w tile's work items (same step, prepended). Slots are
        # released by the w-2 window's readers at runtime (bufs=2).
        for w in range(1, len(XWINS)):
            first_tile = next(n for n, (s0, _) in enumerate(sched)
                              if s0 >= XWINS[w][0])
            at = sched[first_tile - 1][0]
            for k in reversed(range(KC)):
                step_work[at].insert(0, lambda w=w, k=k: dma_xwin(w, k))

        for it in pre_work:
            it()

        # ---------- The scan ----------
        # State nG(t) = S - U(t-1) - X(t)  (U = scaled membrane after reset,
        # X(t) = S*(xp(t)-LEAK) = -nxp(t)).  spk(t) = (R(t-1) >= nG(t)) is a
        # single cheap tensor_tensor is_ge on the critical path (~93ns vs
        # ~190ns for the old fused scalar_tensor_tensor).  The 3 state
        # updates run in the PE burst's shadow:
        #   nG += TH_S*spk(t); nG -= R(t-1); nG += nxp(t+1)
        nc.vector.tensor_scalar(nU[:], nxp[:, 0:32], TH_S, None, alu.add)
        psB_prev = None
        for t in range(t_steps):
            sl = slice(32 * t, 32 * t + 32)
            if t == 0:
                for it in step_work[0]:
                    it()
                nc.vector.tensor_scalar(spk8[:, sl], nU[:], 0.0, None,
                                        alu.is_le)
            else:
                nc.vector.tensor_tensor(spk8[:, sl], psB_prev[:], nU[:],
                                        alu.is_ge)
            if t + 1 < t_steps:
                # ---- PE burst: rec for step t+1 (fp16 x fp8 spikes) ----
                # Emitted BEFORE this step's interleave items so the PE
                # drains phase A matmuls during the sem-hop + spike window
                # instead of queueing them ahead of the burst.
                psB = psB_pool.tile([P, KH * BL], f32, tag="psB", name="psB")
                for j in range(KH):
                    for k in range(KH):
                        nc.tensor.matmul(
                            psB[:, BL * j:BL * (j + 1)],
                            wr_sb[k][:][:, j * P:(j + 1) * P],
                            spk8[:, 32 * t + BL * k:32 * t + BL * (k + 1)],
                            start=(k == 0), stop=(k == KH - 1))
                # ---- state update (off critical path) ----
                nc.vector.scalar_tensor_tensor(nU[:], spk8[:, sl], TH_S,
                                               nU[:], alu.mult, alu.add)
                if psB_prev is not None:
                    nc.vector.tensor_tensor(nU[:], nU[:], psB_prev[:],
                                            alu.subtract)
                nc.vector.tensor_add(nU[:], nU[:],
                                     nxp[:, 32 * (t + 1):32 * (t + 1) + 32])
                psB_prev = psB
            if t > 0:
                for it in step_work[t]:
                    it()
        for tw in range(t_steps, t_steps + 8):
            for it in step_work[tw]:
                it()

        # ---------- Tail: osum accumulated incrementally; just write out.
        if os.environ.get("SNN_FLATC", "0") == "1":
            npad = BL * (t_steps + 1)
            nc.vector.tensor_tensor_scan(v2f[:], d_flat[:], zerf[:], 0.0,
                                         alu.add, alu.max)
            v2f_r = v2f[:].rearrange("o (b tp) -> o b tp", tp=t_steps + 1)
            nc.vector.tensor_reduce(out=osum[:], in_=v2f_r,
                                    axis=mybir.AxisListType.X, op=alu.add)
        nc.sync.dma_start(out=out_d.ap()[:, :], in_=osum[:])

    nc.compile()
    return nc


def prep_inputs(x, w1, w_rec, w2):
    """Build per-core input maps. Host-side transposes/casts (not timed)."""
    from concourse import mybir
    f8np = mybir.dt.np(mybir.dt.float8e4)

    x = np.ascontiguousarray(x, dtype=np.float32)
    w1 = np.ascontiguousarray(w1, dtype=np.float32)
    w_rec = np.ascontiguousarray(w_rec, dtype=np.float32)
    w2 = np.ascontiguousarray(w2, dtype=np.float32)

    w1t = w1.T                                           # [C, H] f32
    w116 = w1t.astype(np.float16)
    wl = w1t - w116.astype(np.float32)
    w1m = (w116.astype(np.float32) * (-SCALE)).astype(np.float16)
    # fp8 correction factors: products = -S * 2^19 * (w1h*xl + wl*xh)
    w8c = np.clip(-SCALE * (2.0 ** SH_W) * w116.astype(np.float32),
                  -240, 240).astype(f8np)
    wl8 = np.clip(-SCALE * (2.0 ** SH_WL) * wl, -240, 240).astype(f8np)
    wdr = np.stack([w8c, wl8], axis=1)                   # [C, 2, H]

    wrt = (w_rec.T * SCALE).astype(np.float16)           # [H, H]
    w2t = np.ascontiguousarray(w2.T).astype(np.float16)  # [H, O]

    in_maps = []
    for c in range(NCORES):
        xc = x[c * BL:(c + 1) * BL]                      # [BL, T, C]
        xt = np.ascontiguousarray(xc.transpose(2, 1, 0).reshape(C, BT))
        xh = xt.astype(np.float16)
        xl = xt - xh.astype(np.float32)
        # pair-sum the correction inputs over adjacent timesteps: the
        # (tiny) correction for steps (2t, 2t+1) is applied at step 2t.
        xl3 = xl.reshape(C, T, BL)
        xh3 = xh.astype(np.float32).reshape(C, T, BL)
        xlp = (xl3[:, 0::2] + xl3[:, 1::2]).reshape(C, BT // 2)
        xhp = (xh3[:, 0::2] + xh3[:, 1::2]).reshape(C, BT // 2)
        xl8 = np.clip(xlp * (2.0 ** SH_XL), -240, 240).astype(f8np)
        x8 = np.clip(xhp * (2.0 ** SH_X), -240, 240).astype(f8np)
        xdr = np.stack([xl8, x8], axis=1)                # [C, 2, BT/2]
        in_maps.append({"xh": xh, "xdr": xdr, "w1m": w1m, "wdr": wdr,
                        "wrt": wrt, "w2t": w2t})
    return in_maps


_LAST = {"exec_time_ns": None, "results": None}


def _setup_trace():
    """Register the axon NTFF profiling hook (works without antenv.axon_hooks
    in the image). Only used when SNN_TRACE=1; safe no-op on failure."""
    try:
        import sys
        import types

        import antenv
        if not hasattr(antenv, "axon_hooks"):
            mod = types.ModuleType("antenv.axon_hooks")
            mod._hook = None
            mod.set_axon_ntff_profile_hook = \
                lambda h: setattr(mod, "_hook", h)
            mod.get_axon_ntff_profile_hook = lambda: mod._hook
            sys.modules["antenv.axon_hooks"] = mod
            antenv.axon_hooks = mod
        if antenv.axon_hooks.get_axon_ntff_profile_hook() is None:
            from trn_agent_boot.trn_boot import _ntff_profile_via_ctypes
            hook = _ntff_profile_via_ctypes('/opt/axon/libaxon_pjrt.so')
            if hook is None:
                return False
            antenv.axon_hooks.set_axon_ntff_profile_hook(hook)
        from concourse import bass_utils
        bass_utils.upload_artifacts = lambda tmpdir: tmpdir
        return True
    except Exception:
        return False


def kernel(x, w1, w_rec, w2):
    from concourse.bass_utils import run_bass_kernel_spmd

    nc = build_nc()
    in_maps = prep_inputs(x, w1, w_rec, w2)
    trace = os.environ.get("SNN_TRACE", "0") == "1" and _setup_trace()
    res = run_bass_kernel_spmd(nc, in_maps, list(range(NCORES)), trace=trace)
    _LAST["exec_time_ns"] = res.exec_time_ns
    _LAST["results"] = res
    out = np.empty((B, O), dtype=np.float32)
    for c in range(NCORES):
        out[c * BL:(c + 1) * BL, :] = res.results[c]["out"].T
    return out



# revision 63
# speedup vs baseline: 1.1633x; 1.1633x over previous
"""Trainium2 Bass kernel for the DVS-SNN problem.

Model (per reference):
    for t in 0..T-1:
        i1 = x_t @ w1.T + spk @ w_rec.T
        v1 = v1 + i1 - LEAK ; spk = (v1 >= 1) ; v1 -= spk
        v2 = max(v2 + spk @ w2.T - OUTPUT_LEAK, 0) ; out_sum += v2
    return out_sum / T

Strategy: data-parallel over batch (64 = 8 cores x 8). Per core, one
interleaved program:
  Warm-up: ~8us of dummy 512-col matmuls release the PE HAM clock-gate
      (default 1.2 GHz) during the initial DMA wait.
  Phase A (time-parallel): nxp[h, 32t+8j+b] = -(S*(xp - LEAK)).
      Main pass: fp16 xh @ (-S*w1h). Correction pass: TWO regular fp8
      matmuls per (k,j) at HALF width -- the tiny cross terms
      (w1h*xl + w1l*xh) are pair-summed over adjacent timesteps on the
      host and applied at even steps (sim rel err 1.19e-2 vs 2e-2 gate).
      Regular fp8 keeps FWL (~25ns hidden LDW); DoubleRow's no-FWL
      LDWEIGHTS would cost a serial ~213ns per (k,j), ~80us of PE.
      Combine on ScalarE (scale) + DVE bias-add / even-add into nxp.
      x streams in 4 windows, double-buffered (xpool bufs=2) on the
      sync/gpsimd queues only -- dma_starts block their issuing queue,
      and sharing with the combine COPYs stalls the strict-FIFO DVE.
  Phase B (sequential scan): state nG = S - U(t-1) - X(t), tiles
      [128, 32]. w_rec fp16 (scaled S=64) stationaries, fp8 spike moving
      (mixed-dtype matmul). Critical path per step (~1us): hop +
      ONE plain tt DVE op spk = (psB >= nG) + hop + the 16 LDW+MM burst
      (25ns NX-floor cadence). Three DVE update ops run in the burst's
      shadow; phase A matmuls fill the hop/spike window on the PE.
  Phase C (per column tile, interleaved): i2 = spk @ w2.T with fp16 w2
      stationary x fp8 spikes; (i2-OL)/T on ScalarE; carry-seeded
      incremental relu-scan + reduce per tile (sentinel resets batch
      lanes), so the tail is just a DMA of osum.
"""

import os
import numpy as np

B, T, C, H, O = 64, 256, 2048, 512, 11
THRESHOLD = 1.0
LEAK = 0.003
OUTPUT_LEAK = LEAK * 0.5

NCORES = 8
BL = B // NCORES          # batch per core = 8
BT = T * BL               # 2048 moving columns per core
P = 128
KC = C // P               # 16 contraction chunks for phase A
KH = H // P               # 4 H chunks
NTILE = 512               # max phase A psum tile (64 timesteps x 8 batch)
XWINS = [(0, 16), (16, 40), (40, 112), (112, 196), (196, 256)]  # x windows
XWMAX = 84                # max window width, timesteps

SCALE = 64.0              # v1 dynamics scale (wrec fp16 subnormal safety)
# fp8 correction-factor scalings (all powers of two; products land at
# -S * 2^CSH * (w1h*xl + w1l*xh)).
SH_W, SH_WL, SH_X, SH_XL = 5, 16, 3, 14
CSH = float(2 ** (SH_W + SH_XL))  # = 2^(SH_WL+SH_X) = 2^19


def _sched(t_steps):
    """Column tiles: ramp from a SMALL first tile (the scan starts once
    tile 0's inputs land -- less startup DMA) to wide tiles. Widths must
    ASCEND (phase C's zero-delta trailing-column trick relies on it) and
    consecutive widths must stay close or the PE queue between bursts
    overloads with the next tile's phase-A matmuls."""
    assert t_steps == 256
    bounds = [0, 16, 40, 72, 112, 152, 196, 256]
    return list(zip(bounds[:-1], bounds[1:]))


def build_nc(t_steps=T):
    """Build the Bass program (same program for all 8 cores)."""
    from contextlib import ExitStack

    import concourse.tile as tile
    from concourse import bacc, mybir

    f32 = mybir.dt.float32
    f16 = mybir.dt.float16
    f8 = mybir.dt.float8e4
    alu = mybir.AluOpType
    ACT = mybir.ActivationFunctionType
    DR = mybir.MatmulPerfMode.DoubleRow

    nc = bacc.Bacc("TRN2", target_bir_lowering=False, debug=False,
                   num_devices=NCORES)

    # ---- DRAM I/O ----
    xh_d = nc.dram_tensor("xh", [C, BT], f16, kind="ExternalInput")
    xdr_d = nc.dram_tensor("xdr", [C, 2, BT // 2], f8, kind="ExternalInput")
    w1m_d = nc.dram_tensor("w1m", [C, H], f16, kind="ExternalInput")
    wdr_d = nc.dram_tensor("wdr", [C, 2, H], f8, kind="ExternalInput")
    wrt_d = nc.dram_tensor("wrt", [H, H], f16, kind="ExternalInput")
    w2t_d = nc.dram_tensor("w2t", [H, O], f16, kind="ExternalInput")
    out_d = nc.dram_tensor("out", [O, BL], f32, kind="ExternalOutput")

    TH_S = SCALE * THRESHOLD
    sched = _sched(t_steps)

    with tile.TileContext(nc) as tc, ExitStack() as ctx:
        perm = ctx.enter_context(tc.tile_pool(name="perm", bufs=1))

        def ptile(shape, dt_, tag):
            return perm.tile(shape, dt_, tag=tag, name=tag)

        nxp = ptile([P, 32 * t_steps], f32, "nxp")          # -(S*(xp-LEAK))
        spk8 = ptile([P, 32 * t_steps], f8, "spk8")         # {0,1} spikes
        nU = ptile([P, 32], f32, "nU")                      # scan state
        # phase C runs INCREMENTALLY per column tile: per batch lane the
        # tile's deltas are scanned with a relu-scan seeded by a carry
        # column (previous tile's final v2); a -inf sentinel column ahead
        # of the carry resets the running value at batch boundaries.
        WMAX = max(b - a for a, b in _sched(t_steps))
        d_c = ptile([O, BL * (WMAX + 2)], f32, "d_c")       # [o,(b,2+w)]
        v2_c = ptile([O, BL * (WMAX + 2)], f32, "v2_c")     # scan output
        zer_c = ptile([O, BL * (WMAX + 2)], f32, "zer_c")
        osum = ptile([O, BL], f32, "osum")
        part = ptile([O, BL], f32, "part")
        if os.environ.get("SNN_FLATC", "0") == "1":
            d_flat = ptile([O, BL * (t_steps + 1)], f32, "d_flat")
            v2f = ptile([O, BL * (t_steps + 1)], f32, "v2f")
            zerf = ptile([O, BL * (t_steps + 1)], f32, "zerf")

        w1m_sb, wdr_sb = [], []
        wr_sb, w2_sb = [], []
        xwin = {}                                            # (k) -> tiles

        # bufs=2: window w+1's DMA streams into the spare slot while
        # window w is still being read -- otherwise the DMA waits on the
        # last reader and the scheduler head-of-line-blocks the PE on an
        # x-gated matmul for ~5us (which also re-throttles the PE clock).
        xpool = ctx.enter_context(tc.tile_pool(name="xp", bufs=2))
        tmp_pool = ctx.enter_context(tc.tile_pool(name="tmpA", bufs=2))
        psA_pool = ctx.enter_context(tc.tile_pool(name="psA", bufs=1,
                                                  space="PSUM"))
        psB_pool = ctx.enter_context(tc.tile_pool(name="psB", bufs=2,
                                                  space="PSUM"))
        psV_pool = ctx.enter_context(tc.tile_pool(name="psV", bufs=2,
                                                  space="PSUM"))

        # Mid-scan x-window DMAs go ONLY on sync/gpsimd: a dma_start
        # blocks its issuing queue on HW-queue/sem waits for ~us; with
        # combine COPYs sharing the scalar queue that stalled the DVE
        # FIFO (combine-ADD between scan state ops) for ~10us per window.
        dma_engines = [nc.sync, nc.gpsimd]
        # Startup-only DMAs may also use scalar (idle pre-scan).
        dma_engines_w = [nc.sync, nc.gpsimd, nc.scalar]

        # ---------- PE HAM warm-up ----------
        # The PE clock-gate defaults to 4/8 (1.2 GHz) and only releases
        # after ~3.4us of sustained PE activity. Without this, the whole
        # phase A ramp for tile 0 runs at half clock while waiting out
        # the DMA stream (~30us wasted). Burn ~6us of dummy LDW+MM pairs
        # during the initial DMA wait (PE would idle anyway).
        # 512-col matmuls keep the PE array continuously busy (tiny MMs
        # have ~50% duty at the NX dispatch floor and never trip the
        # HAM busy-window detector).
        warm = ptile([P, NTILE], f16, "warm")
        nc.vector.memset(warm[:], 0.0)
        warm_ps = psA_pool.tile([P, NTILE], f32, tag="psA0", name="psA",
                                padded_shape=[P, NTILE])
        for _ in range(20):
            nc.tensor.matmul(warm_ps[:], warm[:, 0:P], warm[:],
                             start=True, stop=True)

        def dma_xwin(w, k, eng=None, only=None):
            """Stream x window w for contraction chunk k on a HWDGE queue.
            xdr carries timestep-PAIR-summed correction factors, so its
            column count is half the window's. only='xh'/'xd' issues a
            single tensor (startup orders main-pass inputs first)."""
            w0, w1 = XWINS[w]
            wl = (w1 - w0) * BL
            eng = eng if eng is not None else dma_engines[k % 2]
            csl = slice(k * P, (k + 1) * P)
            if only in (None, 'xh'):
                xh_t = xpool.tile([P, wl], f16, tag=f"xh_{k}",
                                  name=f"xh_{k}",
                                  padded_shape=[P, XWMAX * BL])
                eng.dma_start(out=xh_t[:],
                              in_=xh_d.ap()[csl, w0 * BL:w1 * BL])
                xwin[(w, k, 'h')] = xh_t
            if only in (None, 'xd'):
                xd_t = xpool.tile([P, 2, wl // 2], f8, tag=f"xd_{k}",
                                  name=f"xd_{k}",
                                  padded_shape=[P, 2, XWMAX * BL // 2])
                eng.dma_start(
                    out=xd_t[:],
                    in_=xdr_d.ap()[csl, :, w0 * BL // 2:w1 * BL // 2])
                xwin[(w, k, 'd')] = xd_t

        # ---------- Phase A work for one column tile, as a work list ----------
        # Correction pass: TWO regular fp8 matmuls at HALF width (pair-
        # summed over adjacent timesteps). Regular matmuls keep FWL
        # (~25ns hidden LDW); DoubleRow's 256-col no-FWL LDWEIGHTS costs
        # a serial ~213ns per (k,j) -- ~80us of PE across the scan.
        def a_tile_work(s0, s1):
            ntile = (s1 - s0) * BL
            w = next(i for i, (a, b) in enumerate(XWINS) if a <= s0 < b)
            base = s0 - XWINS[w][0]
            psA = [None] * 2
            psC = [None] * 2
            items = []
            for jp in range(2):
                def alloc(jp=jp):
                    for jj in range(2):
                        psA[jj] = psA_pool.tile([P, ntile], f32,
                                                tag=f"psA{jj}", name="psA",
                                                padded_shape=[P, NTILE])
                        psC[jj] = psA_pool.tile([P, ntile // 2], f32,
                                                tag=f"psC{jj}", name="psC",
                                                padded_shape=[P, NTILE // 2])
                items.append(alloc)
                # main pass first (its inputs stream first at startup),
                # correction pass after
                for k in range(KC):
                    for jj in range(2):
                        def mm_main(k=k, jp=jp, jj=jj):
                            xh_t = xwin[(w, k, 'h')]
                            xsl = slice(base * BL, base * BL + ntile)
                            j = 2 * jp + jj
                            jsl = slice(j * P, (j + 1) * P)
                            nc.tensor.matmul(
                                psA[jj][:], w1m_sb[k][:][:, jsl],
                                xh_t[:, xsl],
                                start=(k == 0), stop=(k == KC - 1))
                        items.append(mm_main)
                for k in range(KC):
                    for jj in range(2):
                        def mm_corr(k=k, jp=jp, jj=jj):
                            xd_t = xwin[(w, k, 'd')]
                            csl = slice(base * BL // 2,
                                        (base * BL + ntile) // 2)
                            j = 2 * jp + jj
                            jsl = slice(j * P, (j + 1) * P)
                            nc.tensor.matmul(
                                psC[jj][:], wdr_sb[k][:][:, 0, jsl],
                                xd_t[:, 0, csl],
                                start=(k == 0), stop=False)
                            nc.tensor.matmul(
                                psC[jj][:], wdr_sb[k][:][:, 1, jsl],
                                xd_t[:, 1, csl],
                                start=False, stop=(k == KC - 1))
                        items.append(mm_corr)
                for jj in range(2):
                    def combine(jj=jj, jp=jp):
                        j = 2 * jp + jj
                        tmp = tmp_pool.tile([P, ntile // 2], f32, tag="tmpA",
                                            name="tmpA",
                                            padded_shape=[P, NTILE // 2])
                        nc.scalar.activation(tmp[:], psC[jj][:], ACT.Copy,
                                             bias=0.0, scale=1.0 / CSH)
                        # nxp = psA + S*LEAK everywhere; pair-summed corr
                        # lands on EVEN timesteps only. Chunked <=16 steps
                        # so the strict-FIFO DVE queue never blocks the
                        # scan's critical compare for long. (Offloading
                        # the odd columns to a strided ScalarE act was a
                        # reproducible +60us regression -- strided PSUM
                        # reads from Act are slow and hold the psA slot.)
                        dest = nxp[:].rearrange(
                            "p (t2 two j b) -> p t2 two j b", two=2,
                            j=KH, b=BL)
                        pA = psA[jj][:].rearrange("p (t2 two b) -> p t2 two b",
                                                  two=2, b=BL)
                        tA = tmp[:].rearrange("p (t2 b) -> p t2 b", b=BL)
                        h0 = s0 // 2
                        nst2 = (s1 - s0) // 2
                        for q0 in range(0, nst2, 8):
                            q1 = min(q0 + 8, nst2)
                            nc.vector.tensor_scalar(
                                dest[:, h0 + q0:h0 + q1, :, j, :],
                                pA[:, q0:q1, :, :], SCALE * LEAK, None,
                                alu.add)
                            nc.vector.tensor_add(
                                dest[:, h0 + q0:h0 + q1, 0, j, :],
                                dest[:, h0 + q0:h0 + q1, 0, j, :],
                                tA[:, q0:q1, :])
                    items.append(combine)
            return items

        # ---------- Phase C work for one column tile (8 spread items) ------
        # Per tile: 4 matmuls -> psV, scale to deltas in d_c cols 2..2+w,
        # carry-seeded relu-scan, reduce, accumulate into osum. The carry
        # chain serializes tile C-work but each link is ~1.5us of DVE.
        def c_tile_items(n, s0, s1):
            spk_r = spk8[:].rearrange("p (t c b) -> p t c b", c=KH, b=BL)
            hold = {}
            w = s1 - s0
            d_r = d_c[:].rearrange("o (b c) -> o b c", c=WMAX + 2)
            v2_r = v2_c[:].rearrange("o (b c) -> o b c", c=WMAX + 2)

            def mk_mm(k):
                def mm(k=k):
                    if k == 0:
                        hold["psV"] = psV_pool.tile(
                            [O, (s1 - s0) * BL], f32, tag="psV",
                            name="psV", padded_shape=[O, NTILE])
                    nc.tensor.matmul(hold["psV"][:], w2_sb[k][:],
                                     spk_r[:, s0:s1, k, :],
                                     start=(k == 0), stop=(k == KH - 1))
                return mm

            def fin():
                # d[o, b, 2+t] = (i2 - OL)/T, written batch-major
                srcv = hold["psV"][:].rearrange("o (t b) -> o b t", b=BL)
                nc.scalar.activation(d_r[:, :, 2:2 + w], srcv, ACT.Copy,
                                     bias=-OUTPUT_LEAK / float(T),
                                     scale=1.0 / float(T))

            def carry():
                # cols 2+w..2+WMAX stay zero-delta (widths ascend), so the
                # previous tile's carry is always at the LAST scan column.
                if n == 0:
                    nc.vector.memset(d_r[:, :, 1:2], 0.0)
                else:
                    nc.vector.tensor_scalar(
                        d_r[:, :, 1:2],
                        v2_r[:, :, WMAX + 1:WMAX + 2],
                        0.0, None, alu.add)

            def scan():
                # full fixed-width flat 2D scan (op requires 2D operands)
                nc.vector.tensor_tensor_scan(v2_c[:], d_c[:], zer_c[:],
                                             0.0, alu.add, alu.max)

            def red():
                nc.vector.tensor_reduce(out=part[:], in_=v2_r[:, :, 2:2 + w],
                                        axis=mybir.AxisListType.X, op=alu.add)
                nc.vector.tensor_add(osum[:], osum[:], part[:])
            if os.environ.get("SNN_FLATC", "0") == "1":
                def fin_flat():
                    dest = d_flat[:].rearrange(
                        "o (b tp) -> o b tp", tp=t_steps + 1)[:, :, s0:s1]
                    srcv = hold["psV"][:].rearrange("o (t b) -> o b t", b=BL)
                    nc.scalar.activation(dest, srcv, ACT.Copy,
                                         bias=-OUTPUT_LEAK / float(T),
                                         scale=1.0 / float(T))
                return [mk_mm(k) for k in range(KH)] + [fin_flat]
            return [mk_mm(k) for k in range(KH)] + [fin, carry, scan, red]

        # ---------- Weight + first-windows DMAs ----------
        # Round-robin each chunk's four tensors across four engine queues
        # (vector/scalar are idle pre-scan) so the serial ~0.65us
        # per-dma_start issue cost is split 4 ways and chunk k's data
        # lands in chunk order. Window 1 preloads into the spare xpool
        # slot (bufs=2) so no x DMA is due during the early scan.
        # Per-chunk interleave: chunk k's four tensors land together in
        # k order, so tile0's PE consumption tracks the DMA stream with
        # no engine-idle gaps (gaps > 3.4us re-throttle the PE clock).
        # Window 1 is NOT loaded here -- it is issued at scan step 0
        # into the spare xpool slot and streams during the early scan.
        for k in range(KC):
            w1m_k = ptile([P, H], f16, f"w1m_{k}")
            wdr_k = ptile([P, 2, H], f8, f"wdr_{k}")
            dma_engines_w[k % 3].dma_start(
                out=w1m_k[:], in_=w1m_d.ap()[k * P:(k + 1) * P, :])
            dma_engines_w[(k + 1) % 3].dma_start(
                out=wdr_k[:], in_=wdr_d.ap()[k * P:(k + 1) * P, :, :])
            w1m_sb.append(w1m_k)
            wdr_sb.append(wdr_k)
            dma_xwin(0, k, eng=dma_engines_w[(k + 2) % 3])
        for k in range(KH):
            wr_k = ptile([P, H], f16, f"wr_{k}")
            nc.sync.dma_start(out=wr_k[:], in_=wrt_d.ap()[k * P:(k + 1) * P, :])
            wr_sb.append(wr_k)
            w2_k = ptile([P, O], f16, f"w2_{k}")
            nc.sync.dma_start(out=w2_k[:], in_=w2t_d.ap()[k * P:(k + 1) * P, :])
            w2_sb.append(w2_k)
        # phase C init: -inf sentinel column per batch lane (resets the
        # relu-scan running value at batch boundaries); delta columns
        # zeroed once (narrow tiles leave trailing columns zero-delta).
        nc.vector.memset(d_c[:], 0.0)
        nc.vector.memset(
            d_c[:].rearrange("o (b c) -> o b c", c=WMAX + 2)[:, :, 0:1],
            -1e30)
        nc.vector.memset(zer_c[:], 0.0)
        nc.vector.memset(osum[:], 0.0)
        if os.environ.get("SNN_FLATC", "0") == "1":
            nc.vector.memset(zerf[:], 0.0)
            nc.vector.memset(
                d_flat[:].rearrange("o (b tp) -> o b tp",
                                    tp=t_steps + 1)[:, :, t_steps:],
                -1e30)

        # ---------- Build per-step interleave schedule ----------
        # Tile n's A work is emitted across the scan steps of tile n-1
        # (disjoint windows: one tile's PSUM accumulation at a time);
        # tiles 0 and 1 are emitted up front.
        step_work = [[] for _ in range(t_steps + 8)]

        def spread(items, lo, hi):
            span = hi - lo
            for i, it in enumerate(items):
                step_work[lo + (i * span) // len(items)].append(it)

        pre_work = []
        for n, (s0, s1) in enumerate(sched):
            items = a_tile_work(s0, s1)
            if n == 0:
                pre_work.extend(items)
            else:
                # End 2 steps before s0: step t's state update READS
                # nxp[t+1], and step_work items are emitted AFTER the
                # scan ops of their step -- a combine landing at step
                # s0-1 would be emitted after the update that reads it
                # (the framework cannot depend on a future writer).
                spread(items, sched[n - 1][0], s0 - 2)
            for i, cit in enumerate(c_tile_items(n, s0, s1)):
                step_work[s1 + i].append(cit)
        # Later x windows: emit window w's DMAs just BEFORE the first
        # window-w tile's work items (same step, prepended). Slots are
        # released by the w-2 window's readers at runtime (bufs=2).
        for w in range(1, len(XWINS)):
            first_tile = next(n for n, (s0, _) in enumerate(sched)
                              if s0 >= XWINS[w][0])
            at = sched[first_tile - 1][0]
            for k in reversed(range(KC)):
                step_work[at].insert(0, lambda w=w, k=k: dma_xwin(w, k))

        for it in pre_work:
            it()

        # ---------- The scan ----------
        # State nG(t) = S - U(t-1) - X(t)  (U = scaled membrane after reset,
        # X(t) = S*(xp(t)-LEAK) = -nxp(t)).  spk(t) = (R(t-1) >= nG(t)) is a
        # single cheap tensor_tensor is_ge on the critical path (~93ns vs
        # ~190ns for the old fused scalar_tensor_tensor).  The 3 state
        # updates run in the PE burst's shadow:
        #   nG += TH_S*spk(t); nG -= R(t-1); nG += nxp(t+1)
        nc.vector.tensor_scalar(nU[:], nxp[:, 0:32], TH_S, None, alu.add)
        psB_prev = None
        for t in range(t_steps):
            sl = slice(32 * t, 32 * t + 32)
            if t == 0:
                for it in step_work[0]:
                    it()
                nc.vector.tensor_scalar(spk8[:, sl], nU[:], 0.0, None,
                                        alu.is_le)
            else:
                nc.vector.tensor_tensor(spk8[:, sl], psB_prev[:], nU[:],
                                        alu.is_ge)
            if t + 1 < t_steps:
                # ---- PE burst: rec for step t+1 (fp16 x fp8 spikes) ----
                # Emitted BEFORE this step's interleave items so the PE
                # drains phase A matmuls during the sem-hop + spike window
                # instead of queueing them ahead of the burst.
                psB = psB_pool.tile([P, KH * BL], f32, tag="psB", name="psB")
                for j in range(KH):
                    for k in range(KH):
                        nc.tensor.matmul(
                            psB[:, BL * j:BL * (j + 1)],
                            wr_sb[k][:][:, j * P:(j + 1) * P],
                            spk8[:, 32 * t + BL * k:32 * t + BL * (k + 1)],
                            start=(k == 0), stop=(k == KH - 1))
                # ---- state update (off critical path) ----
                nc.vector.scalar_tensor_tensor(nU[:], spk8[:, sl], TH_S,
                                               nU[:], alu.mult, alu.add)
                if psB_prev is not None:
                    nc.vector.tensor_tensor(nU[:], nU[:], psB_prev[:],
                                            alu.subtract)
                nc.vector.tensor_add(nU[:], nU[:],
                                     nxp[:, 32 * (t + 1):32 * (t + 1) + 32])
                psB_prev = psB
            if t > 0:
                for it in step_work[t]:
                    it()
        for tw in range(t_steps, t_steps + 8):
            for it in step_work[tw]:
                it()

        # ---------- Tail: osum accumulated incrementally; just write out.
        if os.environ.get("SNN_FLATC", "0") == "1":
            npad = BL * (t_steps + 1)
            nc.vector.tensor_tensor_scan(v2f[:], d_flat[:], zerf[:], 0.0,
                                         alu.add, alu.max)
            v2f_r = v2f[:].rearrange("o (b tp) -> o b tp", tp=t_steps + 1)
            nc.vector.tensor_reduce(out=osum[:], in_=v2f_r,
                                    axis=mybir.AxisListType.X, op=alu.add)
        nc.sync.dma_start(out=out_d.ap()[:, :], in_=osum[:])

    nc.compile()
    return nc


def prep_inputs(x, w1, w_rec, w2):
    """Build per-core input maps. Host-side transposes/casts (not timed)."""
    from concourse import mybir
    f8np = mybir.dt.np(mybir.dt.float8e4)

    x = np.ascontiguousarray(x, dtype=np.float32)
    w1 = np.ascontiguousarray(w1, dtype=np.float32)
    w_rec = np.ascontiguousarray(w_rec, dtype=np.float32)
    w2 = np.ascontiguousarray(w2, dtype=np.float32)

    w1t = w1.T                                           # [C, H] f32
    w116 = w1t.astype(np.float16)
    wl = w1t - w116.astype(np.float32)
    w1m = (w116.astype(np.float32) * (-SCALE)).astype(np.float16)
    # fp8 correction factors: products = -S * 2^19 * (w1h*xl + wl*xh)
    w8c = np.clip(-SCALE * (2.0 ** SH_W) * w116.astype(np.float32),
                  -240, 240).astype(f8np)
    wl8 = np.clip(-SCALE * (2.0 ** SH_WL) * wl, -240, 240).astype(f8np)
    wdr = np.stack([w8c, wl8], axis=1)                   # [C, 2, H]

    wrt = (w_rec.T * SCALE).astype(np.float16)           # [H, H]
    w2t = np.ascontiguousarray(w2.T).astype(np.float16)  # [H, O]

    in_maps = []
    for c in range(NCORES):
        xc = x[c * BL:(c + 1) * BL]                      # [BL, T, C]
        xt = np.ascontiguousarray(xc.transpose(2, 1, 0).reshape(C, BT))
        xh = xt.astype(np.float16)
        xl = xt - xh.astype(np.float32)
        # pair-sum the correction inputs over adjacent timesteps: the
        # (tiny) correction for steps (2t, 2t+1) is applied at step 2t.
        xl3 = xl.reshape(C, T, BL)
        xh3 = xh.astype(np.float32).reshape(C, T, BL)
        xlp = (xl3[:, 0::2] + xl3[:, 1::2]).reshape(C, BT // 2)
        xhp = (xh3[:, 0::2] + xh3[:, 1::2]).reshape(C, BT // 2)
        xl8 = np.clip(xlp * (2.0 ** SH_XL), -240, 240).astype(f8np)
        x8 = np.clip(xhp * (2.0 ** SH_X), -240, 240).astype(f8np)
        xdr = np.stack([xl8, x8], axis=1)                # [C, 2, BT/2]
        in_maps.append({"xh": xh, "xdr": xdr, "w1m": w1m, "wdr": wdr,
                        "wrt": wrt, "w2t": w2t})
    return in_maps


_LAST = {"exec_time_ns": None, "results": None}


def _setup_trace():
    """Register the axon NTFF profiling hook (works without antenv.axon_hooks
    in the image). Only used when SNN_TRACE=1; safe no-op on failure."""
    try:
        import sys
        import types

        import antenv
        if not hasattr(antenv, "axon_hooks"):
            mod = types.ModuleType("antenv.axon_hooks")
            mod._hook = None
            mod.set_axon_ntff_profile_hook = \
                lambda h: setattr(mod, "_hook", h)
            mod.get_axon_ntff_profile_hook = lambda: mod._hook
            sys.modules["antenv.axon_hooks"] = mod
            antenv.axon_hooks = mod
        if antenv.axon_hooks.get_axon_ntff_profile_hook() is None:
            from trn_agent_boot.trn_boot import _ntff_profile_via_ctypes
            hook = _ntff_profile_via_ctypes('/opt/axon/libaxon_pjrt.so')
            if hook is None:
                return False
            antenv.axon_hooks.set_axon_ntff_profile_hook(hook)
        from concourse import bass_utils
        bass_utils.upload_artifacts = lambda tmpdir: tmpdir
        return True
    except Exception:
        return False


def kernel(x, w1, w_rec, w2):
    from concourse.bass_utils import run_bass_kernel_spmd

    nc = build_nc()
    in_maps = prep_inputs(x, w1, w_rec, w2)
    trace = os.environ.get("SNN_TRACE", "0") == "1" and _setup_trace()
    res = run_bass_kernel_spmd(nc, in_maps, list(range(NCORES)), trace=trace)
    _LAST["exec_time_ns"] = res.exec_time_ns
    _LAST["results"] = res
    out = np.empty((B, O), dtype=np.float32)
    for c in range(NCORES):
        out[c * BL:(c + 1) * BL, :] = res.results[c]["out"].T
    return out



# revision 64
# speedup vs baseline: 1.1670x; 1.0032x over previous
"""Trainium2 Bass kernel for the DVS-SNN problem.

Model (per reference):
    for t in 0..T-1:
        i1 = x_t @ w1.T + spk @ w_rec.T
        v1 = v1 + i1 - LEAK ; spk = (v1 >= 1) ; v1 -= spk
        v2 = max(v2 + spk @ w2.T - OUTPUT_LEAK, 0) ; out_sum += v2
    return out_sum / T

Strategy: data-parallel over batch (64 = 8 cores x 8). Per core, one
interleaved program:
  Warm-up: ~8us of dummy 512-col matmuls release the PE HAM clock-gate
      (default 1.2 GHz) during the initial DMA wait.
  Phase A (time-parallel): nxp[h, 32t+8j+b] = -(S*(xp - LEAK)).
      Main pass: fp16 xh @ (-S*w1h). Correction pass: TWO regular fp8
      matmuls per (k,j) at HALF width -- the tiny cross terms
      (w1h*xl + w1l*xh) are pair-summed over adjacent timesteps on the
      host and applied at even steps (sim rel err 1.19e-2 vs 2e-2 gate).
      Regular fp8 keeps FWL (~25ns hidden LDW); DoubleRow's no-FWL
      LDWEIGHTS would cost a serial ~213ns per (k,j), ~80us of PE.
      Combine on ScalarE (scale) + DVE bias-add / even-add into nxp.
      x streams in 4 windows, double-buffered (xpool bufs=2) on the
      sync/gpsimd queues only -- dma_starts block their issuing queue,
      and sharing with the combine COPYs stalls the strict-FIFO DVE.
  Phase B (sequential scan): state nG = S - U(t-1) - X(t), tiles
      [128, 32]. w_rec fp16 (scaled S=64) stationaries, fp8 spike moving
      (mixed-dtype matmul). Critical path per step (~1us): hop +
      ONE plain tt DVE op spk = (psB >= nG) + hop + the 16 LDW+MM burst
      (25ns NX-floor cadence). Three DVE update ops run in the burst's
      shadow; phase A matmuls fill the hop/spike window on the PE.
  Phase C (per column tile, interleaved): i2 = spk @ w2.T with fp16 w2
      stationary x fp8 spikes; (i2-OL)/T on ScalarE; carry-seeded
      incremental relu-scan + reduce per tile (sentinel resets batch
      lanes), so the tail is just a DMA of osum.
"""

import os
import numpy as np

B, T, C, H, O = 64, 256, 2048, 512, 11
THRESHOLD = 1.0
LEAK = 0.003
OUTPUT_LEAK = LEAK * 0.5

NCORES = 8
BL = B // NCORES          # batch per core = 8
BT = T * BL               # 2048 moving columns per core
P = 128
KC = C // P               # 16 contraction chunks for phase A
KH = H // P               # 4 H chunks
NTILE = 512               # max phase A psum tile (64 timesteps x 8 batch)
XWINS = [(0, 16), (16, 40), (40, 112), (112, 196), (196, 256)]  # x windows
XWMAX = 84                # max window width, timesteps

SCALE = 64.0              # v1 dynamics scale (wrec fp16 subnormal safety)
# fp8 correction-factor scalings (all powers of two; products land at
# -S * 2^CSH * (w1h*xl + w1l*xh)).
SH_W, SH_WL, SH_X, SH_XL = 5, 16, 3, 14
CSH = float(2 ** (SH_W + SH_XL))  # = 2^(SH_WL+SH_X) = 2^19


def _sched(t_steps):
    """Column tiles: ramp from a SMALL first tile (the scan starts once
    tile 0's inputs land -- less startup DMA) to wide tiles. Widths must
    ASCEND (phase C's zero-delta trailing-column trick relies on it) and
    consecutive widths must stay close or the PE queue between bursts
    overloads with the next tile's phase-A matmuls."""
    assert t_steps == 256
    bounds = [0, 16, 40, 72, 112, 152, 196, 256]
    return list(zip(bounds[:-1], bounds[1:]))


def build_nc(t_steps=T):
    """Build the Bass program (same program for all 8 cores)."""
    from contextlib import ExitStack

    import concourse.tile as tile
    from concourse import bacc, mybir

    f32 = mybir.dt.float32
    f16 = mybir.dt.float16
    f8 = mybir.dt.float8e4
    alu = mybir.AluOpType
    ACT = mybir.ActivationFunctionType
    DR = mybir.MatmulPerfMode.DoubleRow

    nc = bacc.Bacc("TRN2", target_bir_lowering=False, debug=False,
                   num_devices=NCORES)

    # ---- DRAM I/O ----
    xh_d = nc.dram_tensor("xh", [C, BT], f16, kind="ExternalInput")
    xdr_d = nc.dram_tensor("xdr", [C, 2, BT // 2], f8, kind="ExternalInput")
    w1m_d = nc.dram_tensor("w1m", [C, H], f16, kind="ExternalInput")
    wdr_d = nc.dram_tensor("wdr", [C, 2, H], f8, kind="ExternalInput")
    wrt_d = nc.dram_tensor("wrt", [H, H], f16, kind="ExternalInput")
    w2t_d = nc.dram_tensor("w2t", [H, O], f16, kind="ExternalInput")
    out_d = nc.dram_tensor("out", [O, BL], f32, kind="ExternalOutput")

    TH_S = SCALE * THRESHOLD
    sched = _sched(t_steps)

    with tile.TileContext(nc) as tc, ExitStack() as ctx:
        perm = ctx.enter_context(tc.tile_pool(name="perm", bufs=1))

        def ptile(shape, dt_, tag):
            return perm.tile(shape, dt_, tag=tag, name=tag)

        nxp = ptile([P, 32 * t_steps], f32, "nxp")          # -(S*(xp-LEAK))
        spk8 = ptile([P, 32 * t_steps], f8, "spk8")         # {0,1} spikes
        nU = ptile([P, 32], f32, "nU")                      # scan state
        # phase C runs INCREMENTALLY per column tile: per batch lane the
        # tile's deltas are scanned with a relu-scan seeded by a carry
        # column (previous tile's final v2); a -inf sentinel column ahead
        # of the carry resets the running value at batch boundaries.
        WMAX = max(b - a for a, b in _sched(t_steps))
        d_c = ptile([O, BL * (WMAX + 2)], f32, "d_c")       # [o,(b,2+w)]
        v2_c = ptile([O, BL * (WMAX + 2)], f32, "v2_c")     # scan output
        zer_c = ptile([O, BL * (WMAX + 2)], f32, "zer_c")
        osum = ptile([O, BL], f32, "osum")
        part = ptile([O, BL], f32, "part")
        if os.environ.get("SNN_FLATC", "0") == "1":
            d_flat = ptile([O, BL * (t_steps + 1)], f32, "d_flat")
            v2f = ptile([O, BL * (t_steps + 1)], f32, "v2f")
            zerf = ptile([O, BL * (t_steps + 1)], f32, "zerf")

        w1m_sb, wdr_sb = [], []
        wr_sb, w2_sb = [], []
        xwin = {}                                            # (k) -> tiles

        # bufs=2: window w+1's DMA streams into the spare slot while
        # window w is still being read -- otherwise the DMA waits on the
        # last reader and the scheduler head-of-line-blocks the PE on an
        # x-gated matmul for ~5us (which also re-throttles the PE clock).
        xpool = ctx.enter_context(tc.tile_pool(name="xp", bufs=2))
        tmp_pool = ctx.enter_context(tc.tile_pool(name="tmpA", bufs=2))
        psA_pool = ctx.enter_context(tc.tile_pool(name="psA", bufs=1,
                                                  space="PSUM"))
        psB_pool = ctx.enter_context(tc.tile_pool(name="psB", bufs=2,
                                                  space="PSUM"))
        psV_pool = ctx.enter_context(tc.tile_pool(name="psV", bufs=2,
                                                  space="PSUM"))

        # Mid-scan x-window DMAs go ONLY on sync/gpsimd: a dma_start
        # blocks its issuing queue on HW-queue/sem waits for ~us; with
        # combine COPYs sharing the scalar queue that stalled the DVE
        # FIFO (combine-ADD between scan state ops) for ~10us per window.
        dma_engines = [nc.sync, nc.gpsimd]
        # Startup-only DMAs may also use scalar (idle pre-scan).
        dma_engines_w = [nc.sync, nc.gpsimd, nc.scalar]

        # ---------- PE HAM warm-up ----------
        # The PE clock-gate defaults to 4/8 (1.2 GHz) and only releases
        # after ~3.4us of sustained PE activity. Without this, the whole
        # phase A ramp for tile 0 runs at half clock while waiting out
        # the DMA stream (~30us wasted). Burn ~6us of dummy LDW+MM pairs
        # during the initial DMA wait (PE would idle anyway).
        # 512-col matmuls keep the PE array continuously busy (tiny MMs
        # have ~50% duty at the NX dispatch floor and never trip the
        # HAM busy-window detector).
        warm = ptile([P, NTILE], f16, "warm")
        nc.vector.memset(warm[:], 0.0)
        warm_ps = psA_pool.tile([P, NTILE], f32, tag="psA0", name="psA",
                                padded_shape=[P, NTILE])
        for _ in range(20):
            nc.tensor.matmul(warm_ps[:], warm[:, 0:P], warm[:],
                             start=True, stop=True)

        def dma_xwin(w, k, eng=None, only=None):
            """Stream x window w for contraction chunk k on a HWDGE queue.
            xdr carries timestep-PAIR-summed correction factors, so its
            column count is half the window's. only='xh'/'xd' issues a
            single tensor (startup orders main-pass inputs first)."""
            w0, w1 = XWINS[w]
            wl = (w1 - w0) * BL
            eng = eng if eng is not None else dma_engines[k % 2]
            csl = slice(k * P, (k + 1) * P)
            if only in (None, 'xh'):
                xh_t = xpool.tile([P, wl], f16, tag=f"xh_{k}",
                                  name=f"xh_{k}",
                                  padded_shape=[P, XWMAX * BL])
                eng.dma_start(out=xh_t[:],
                              in_=xh_d.ap()[csl, w0 * BL:w1 * BL])
                xwin[(w, k, 'h')] = xh_t
            if only in (None, 'xd'):
                xd_t = xpool.tile([P, 2, wl // 2], f8, tag=f"xd_{k}",
                                  name=f"xd_{k}",
                                  padded_shape=[P, 2, XWMAX * BL // 2])
                eng.dma_start(
                    out=xd_t[:],
                    in_=xdr_d.ap()[csl, :, w0 * BL // 2:w1 * BL // 2])
                xwin[(w, k, 'd')] = xd_t

        # ---------- Phase A work for one column tile, as a work list ----------
        # Correction pass: TWO regular fp8 matmuls at HALF width (pair-
        # summed over adjacent timesteps). Regular matmuls keep FWL
        # (~25ns hidden LDW); DoubleRow's 256-col no-FWL LDWEIGHTS costs
        # a serial ~213ns per (k,j) -- ~80us of PE across the scan.
        def a_tile_work(s0, s1):
            ntile = (s1 - s0) * BL
            w = next(i for i, (a, b) in enumerate(XWINS) if a <= s0 < b)
            base = s0 - XWINS[w][0]
            psA = [None] * 2
            psC = [None] * 2
            items = []
            for jp in range(2):
                def alloc(jp=jp):
                    for jj in range(2):
                        psA[jj] = psA_pool.tile([P, ntile], f32,
                                                tag=f"psA{jj}", name="psA",
                                                padded_shape=[P, NTILE])
                        psC[jj] = psA_pool.tile([P, ntile // 2], f32,
                                                tag=f"psC{jj}", name="psC",
                                                padded_shape=[P, NTILE // 2])
                items.append(alloc)
                # main pass first (its inputs stream first at startup),
                # correction pass after
                for k in range(KC):
                    for jj in range(2):
                        def mm_main(k=k, jp=jp, jj=jj):
                            xh_t = xwin[(w, k, 'h')]
                            xsl = slice(base * BL, base * BL + ntile)
                            j = 2 * jp + jj
                            jsl = slice(j * P, (j + 1) * P)
                            nc.tensor.matmul(
                                psA[jj][:], w1m_sb[k][:][:, jsl],
                                xh_t[:, xsl],
                                start=(k == 0), stop=(k == KC - 1))
                        items.append(mm_main)
                for k in range(KC):
                    for jj in range(2):
                        def mm_corr(k=k, jp=jp, jj=jj):
                            xd_t = xwin[(w, k, 'd')]
                            csl = slice(base * BL // 2,
                                        (base * BL + ntile) // 2)
                            j = 2 * jp + jj
                            jsl = slice(j * P, (j + 1) * P)
                            nc.tensor.matmul(
                                psC[jj][:], wdr_sb[k][:][:, 0, jsl],
                                xd_t[:, 0, csl],
                                start=(k == 0), stop=False)
                            nc.tensor.matmul(
                                psC[jj][:], wdr_sb[k][:][:, 1, jsl],
                                xd_t[:, 1, csl],
                                start=False, stop=(k == KC - 1))
                        items.append(mm_corr)
                for jj in range(2):
                    def combine(jj=jj, jp=jp):
                        j = 2 * jp + jj
                        tmp = tmp_pool.tile([P, ntile // 2], f32, tag="tmpA",
                                            name="tmpA",
                                            padded_shape=[P, NTILE // 2])
                        nc.scalar.activation(tmp[:], psC[jj][:], ACT.Copy,
                                             bias=0.0, scale=1.0 / CSH)
                        # nxp = psA + S*LEAK everywhere; pair-summed corr
                        # lands on EVEN timesteps only. Chunked <=16 steps
                        # so the strict-FIFO DVE queue never blocks the
                        # scan's critical compare for long. (Offloading
                        # the odd columns to a strided ScalarE act was a
                        # reproducible +60us regression -- strided PSUM
                        # reads from Act are slow and hold the psA slot.)
                        dest = nxp[:].rearrange(
                            "p (t2 two j b) -> p t2 two j b", two=2,
                            j=KH, b=BL)
                        pA = psA[jj][:].rearrange("p (t2 two b) -> p t2 two b",
                                                  two=2, b=BL)
                        tA = tmp[:].rearrange("p (t2 b) -> p t2 b", b=BL)
                        h0 = s0 // 2
                        nst2 = (s1 - s0) // 2
                        for q0 in range(0, nst2, 8):
                            q1 = min(q0 + 8, nst2)
                            nc.vector.tensor_scalar(
                                dest[:, h0 + q0:h0 + q1, :, j, :],
                                pA[:, q0:q1, :, :], SCALE * LEAK, None,
                                alu.add)
                            nc.vector.tensor_add(
                                dest[:, h0 + q0:h0 + q1, 0, j, :],
                                dest[:, h0 + q0:h0 + q1, 0, j, :],
                                tA[:, q0:q1, :])
                    items.append(combine)
            return items

        # ---------- Phase C work for one column tile (8 spread items) ------
        # Per tile: 4 matmuls -> psV, scale to deltas in d_c cols 2..2+w,
        # carry-seeded relu-scan, reduce, accumulate into osum. The carry
        # chain serializes tile C-work but each link is ~1.5us of DVE.
        def c_tile_items(n, s0, s1):
            spk_r = spk8[:].rearrange("p (t c b) -> p t c b", c=KH, b=BL)
            hold = {}
            w = s1 - s0
            d_r = d_c[:].rearrange("o (b c) -> o b c", c=WMAX + 2)
            v2_r = v2_c[:].rearrange("o (b c) -> o b c", c=WMAX + 2)

            def mk_mm(k):
                def mm(k=k):
                    if k == 0:
                        hold["psV"] = psV_pool.tile(
                            [O, (s1 - s0) * BL], f32, tag="psV",
                            name="psV", padded_shape=[O, NTILE])
                    nc.tensor.matmul(hold["psV"][:], w2_sb[k][:],
                                     spk_r[:, s0:s1, k, :],
                                     start=(k == 0), stop=(k == KH - 1))
                return mm

            def fin():
                # d[o, b, 2+t] = (i2 - OL)/T, written batch-major
                srcv = hold["psV"][:].rearrange("o (t b) -> o b t", b=BL)
                nc.scalar.activation(d_r[:, :, 2:2 + w], srcv, ACT.Copy,
                                     bias=-OUTPUT_LEAK / float(T),
                                     scale=1.0 / float(T))

            def carry():
                # cols 2+w..2+WMAX stay zero-delta (widths ascend), so the
                # previous tile's carry is always at the LAST scan column.
                if n == 0:
                    nc.vector.memset(d_r[:, :, 1:2], 0.0)
                else:
                    nc.vector.tensor_scalar(
                        d_r[:, :, 1:2],
                        v2_r[:, :, WMAX + 1:WMAX + 2],
                        0.0, None, alu.add)

            def scan():
                # full fixed-width flat 2D scan (op requires 2D operands)
                nc.vector.tensor_tensor_scan(v2_c[:], d_c[:], zer_c[:],
                                             0.0, alu.add, alu.max)

            def red():
                nc.vector.tensor_reduce(out=part[:], in_=v2_r[:, :, 2:2 + w],
                                        axis=mybir.AxisListType.X, op=alu.add)
                nc.vector.tensor_add(osum[:], osum[:], part[:])
            if os.environ.get("SNN_FLATC", "0") == "1":
                def fin_flat():
                    dest = d_flat[:].rearrange(
                        "o (b tp) -> o b tp", tp=t_steps + 1)[:, :, s0:s1]
                    srcv = hold["psV"][:].rearrange("o (t b) -> o b t", b=BL)
                    nc.scalar.activation(dest, srcv, ACT.Copy,
                                         bias=-OUTPUT_LEAK / float(T),
                                         scale=1.0 / float(T))
                return [mk_mm(k) for k in range(KH)] + [fin_flat]
            return [mk_mm(k) for k in range(KH)] + [fin, carry, scan, red]

        # ---------- Weight + first-windows DMAs ----------
        # Round-robin each chunk's four tensors across four engine queues
        # (vector/scalar are idle pre-scan) so the serial ~0.65us
        # per-dma_start issue cost is split 4 ways and chunk k's data
        # lands in chunk order. Window 1 preloads into the spare xpool
        # slot (bufs=2) so no x DMA is due during the early scan.
        # Per-chunk interleave: chunk k's four tensors land together in
        # k order, so tile0's PE consumption tracks the DMA stream with
        # no engine-idle gaps (gaps > 3.4us re-throttle the PE clock).
        # Window 1 is NOT loaded here -- it is issued at scan step 0
        # into the spare xpool slot and streams during the early scan.
        for k in range(KC):
            w1m_k = ptile([P, H], f16, f"w1m_{k}")
            wdr_k = ptile([P, 2, H], f8, f"wdr_{k}")
            dma_engines_w[k % 3].dma_start(
                out=w1m_k[:], in_=w1m_d.ap()[k * P:(k + 1) * P, :])
            dma_engines_w[(k + 1) % 3].dma_start(
                out=wdr_k[:], in_=wdr_d.ap()[k * P:(k + 1) * P, :, :])
            w1m_sb.append(w1m_k)
            wdr_sb.append(wdr_k)
            dma_xwin(0, k, eng=dma_engines_w[(k + 2) % 3])
        for k in range(KH):
            wr_k = ptile([P, H], f16, f"wr_{k}")
            nc.sync.dma_start(out=wr_k[:], in_=wrt_d.ap()[k * P:(k + 1) * P, :])
            wr_sb.append(wr_k)
            w2_k = ptile([P, O], f16, f"w2_{k}")
            nc.sync.dma_start(out=w2_k[:], in_=w2t_d.ap()[k * P:(k + 1) * P, :])
            w2_sb.append(w2_k)
        # phase C init: -inf sentinel column per batch lane (resets the
        # relu-scan running value at batch boundaries); delta columns
        # zeroed once (narrow tiles leave trailing columns zero-delta).
        nc.vector.memset(d_c[:], 0.0)
        nc.vector.memset(
            d_c[:].rearrange("o (b c) -> o b c", c=WMAX + 2)[:, :, 0:1],
            -1e30)
        nc.vector.memset(zer_c[:], 0.0)
        nc.vector.memset(osum[:], 0.0)
        if os.environ.get("SNN_FLATC", "0") == "1":
            nc.vector.memset(zerf[:], 0.0)
            nc.vector.memset(
                d_flat[:].rearrange("o (b tp) -> o b tp",
                                    tp=t_steps + 1)[:, :, t_steps:],
                -1e30)

        # ---------- Build per-step interleave schedule ----------
        # Tile n's A work is emitted across the scan steps of tile n-1
        # (disjoint windows: one tile's PSUM accumulation at a time);
        # tiles 0 and 1 are emitted up front.
        step_work = [[] for _ in range(t_steps + 8)]

        def spread(items, lo, hi):
            span = hi - lo
            for i, it in enumerate(items):
                step_work[lo + (i * span) // len(items)].append(it)

        pre_work = []
        for n, (s0, s1) in enumerate(sched):
            items = a_tile_work(s0, s1)
            if n == 0:
                pre_work.extend(items)
            else:
                # End 2 steps before s0: step t's state update READS
                # nxp[t+1], and step_work items are emitted AFTER the
                # scan ops of their step -- a combine landing at step
                # s0-1 would be emitted after the update that reads it
                # (the framework cannot depend on a future writer).
                spread(items, sched[n - 1][0], s0 - 2)
            # Phase C items spaced 3 steps apart (except the tail tile):
            # bunching 4 slow psV matmuls + 4 DVE ops onto consecutive
            # steps right at the tile boundary -- where the next tile's
            # A-work also starts -- made those steps 1.5-3x slow.
            csp = 3 if s1 < t_steps else 1
            for i, cit in enumerate(c_tile_items(n, s0, s1)):
                step_work[s1 + csp * i].append(cit)
        # Later x windows: emit window w's DMAs just BEFORE the first
        # window-w tile's work items (same step, prepended). Slots are
        # released by the w-2 window's readers at runtime (bufs=2).
        for w in range(1, len(XWINS)):
            first_tile = next(n for n, (s0, _) in enumerate(sched)
                              if s0 >= XWINS[w][0])
            at = sched[first_tile - 1][0]
            for k in reversed(range(KC)):
                step_work[at].insert(0, lambda w=w, k=k: dma_xwin(w, k))

        for it in pre_work:
            it()

        # ---------- The scan ----------
        # State nG(t) = S - U(t-1) - X(t)  (U = scaled membrane after reset,
        # X(t) = S*(xp(t)-LEAK) = -nxp(t)).  spk(t) = (R(t-1) >= nG(t)) is a
        # single cheap tensor_tensor is_ge on the critical path (~93ns vs
        # ~190ns for the old fused scalar_tensor_tensor).  The 3 state
        # updates run in the PE burst's shadow:
        #   nG += TH_S*spk(t); nG -= R(t-1); nG += nxp(t+1)
        nc.vector.tensor_scalar(nU[:], nxp[:, 0:32], TH_S, None, alu.add)
        psB_prev = None
        for t in range(t_steps):
            sl = slice(32 * t, 32 * t + 32)
            if t == 0:
                for it in step_work[0]:
                    it()
                nc.vector.tensor_scalar(spk8[:, sl], nU[:], 0.0, None,
                                        alu.is_le)
            else:
                nc.vector.tensor_tensor(spk8[:, sl], psB_prev[:], nU[:],
                                        alu.is_ge)
            if t + 1 < t_steps:
                # ---- PE burst: rec for step t+1 (fp16 x fp8 spikes) ----
                # Emitted BEFORE this step's interleave items so the PE
                # drains phase A matmuls during the sem-hop + spike window
                # instead of queueing them ahead of the burst.
                psB = psB_pool.tile([P, KH * BL], f32, tag="psB", name="psB")
                for j in range(KH):
                    for k in range(KH):
                        nc.tensor.matmul(
                            psB[:, BL * j:BL * (j + 1)],
                            wr_sb[k][:][:, j * P:(j + 1) * P],
                            spk8[:, 32 * t + BL * k:32 * t + BL * (k + 1)],
                            start=(k == 0), stop=(k == KH - 1))
                # ---- state update (off critical path) ----
                nc.vector.scalar_tensor_tensor(nU[:], spk8[:, sl], TH_S,
                                               nU[:], alu.mult, alu.add)
                if psB_prev is not None:
                    nc.vector.tensor_tensor(nU[:], nU[:], psB_prev[:],
                                            alu.subtract)
                nc.vector.tensor_add(nU[:], nU[:],
                                     nxp[:, 32 * (t + 1):32 * (t + 1) + 32])
                psB_prev = psB
            if t > 0:
                for it in step_work[t]:
                    it()
        for tw in range(t_steps, t_steps + 8):
            for it in step_work[tw]:
                it()

        # ---------- Tail: osum accumulated incrementally; just write out.
        if os.environ.get("SNN_FLATC", "0") == "1":
            npad = BL * (t_steps + 1)
            nc.vector.tensor_tensor_scan(v2f[:], d_flat[:], zerf[:], 0.0,
                                         alu.add, alu.max)
            v2f_r = v2f[:].rearrange("o (b tp) -> o b tp", tp=t_steps + 1)
            nc.vector.tensor_reduce(out=osum[:], in_=v2f_r,
                                    axis=mybir.AxisListType.X, op=alu.add)
        nc.sync.dma_start(out=out_d.ap()[:, :], in_=osum[:])

    nc.compile()
    return nc


def prep_inputs(x, w1, w_rec, w2):
    """Build per-core input maps. Host-side transposes/casts (not timed)."""
    from concourse import mybir
    f8np = mybir.dt.np(mybir.dt.float8e4)

    x = np.ascontiguousarray(x, dtype=np.float32)
    w1 = np.ascontiguousarray(w1, dtype=np.float32)
    w_rec = np.ascontiguousarray(w_rec, dtype=np.float32)
    w2 = np.ascontiguousarray(w2, dtype=np.float32)

    w1t = w1.T                                           # [C, H] f32
    w116 = w1t.astype(np.float16)
    wl = w1t - w116.astype(np.float32)
    w1m = (w116.astype(np.float32) * (-SCALE)).astype(np.float16)
    # fp8 correction factors: products = -S * 2^19 * (w1h*xl + wl*xh)
    w8c = np.clip(-SCALE * (2.0 ** SH_W) * w116.astype(np.float32),
                  -240, 240).astype(f8np)
    wl8 = np.clip(-SCALE * (2.0 ** SH_WL) * wl, -240, 240).astype(f8np)
    wdr = np.stack([w8c, wl8], axis=1)                   # [C, 2, H]

    wrt = (w_rec.T * SCALE).astype(np.float16)           # [H, H]
    w2t = np.ascontiguousarray(w2.T).astype(np.float16)  # [H, O]

    in_maps = []
    for c in range(NCORES):
        xc = x[c * BL:(c + 1) * BL]                      # [BL, T, C]
        xt = np.ascontiguousarray(xc.transpose(2, 1, 0).reshape(C, BT))
        xh = xt.astype(np.float16)
        xl = xt - xh.astype(np.float32)
        # pair-sum the correction inputs over adjacent timesteps: the
        # (tiny) correction for steps (2t, 2t+1) is applied at step 2t.
        xl3 = xl.reshape(C, T, BL)
        xh3 = xh.astype(np.float32).reshape(C, T, BL)
        xlp = (xl3[:, 0::2] + xl3[:, 1::2]).reshape(C, BT // 2)
        xhp = (xh3[:, 0::2] + xh3[:, 1::2]).reshape(C, BT // 2)
        xl8 = np.clip(xlp * (2.0 ** SH_XL), -240, 240).astype(f8np)
        x8 = np.clip(xhp * (2.0 ** SH_X), -240, 240).astype(f8np)
        xdr = np.stack([xl8, x8], axis=1)                # [C, 2, BT/2]
        in_maps.append({"xh": xh, "xdr": xdr, "w1m": w1m, "wdr": wdr,
                        "wrt": wrt, "w2t": w2t})
    return in_maps


_LAST = {"exec_time_ns": None, "results": None}


def _setup_trace():
    """Register the axon NTFF profiling hook (works without antenv.axon_hooks
    in the image). Only used when SNN_TRACE=1; safe no-op on failure."""
    try:
        import sys
        import types

        import antenv
        if not hasattr(antenv, "axon_hooks"):
            mod = types.ModuleType("antenv.axon_hooks")
            mod._hook = None
            mod.set_axon_ntff_profile_hook = \
                lambda h: setattr(mod, "_hook", h)
            mod.get_axon_ntff_profile_hook = lambda: mod._hook
            sys.modules["antenv.axon_hooks"] = mod
            antenv.axon_hooks = mod
        if antenv.axon_hooks.get_axon_ntff_profile_hook() is None:
            from trn_agent_boot.trn_boot import _ntff_profile_via_ctypes
            hook = _ntff_profile_via_ctypes('/opt/axon/libaxon_pjrt.so')
            if hook is None:
                return False
            antenv.axon_hooks.set_axon_ntff_profile_hook(hook)
        from concourse import bass_utils
        bass_utils.upload_artifacts = lambda tmpdir: tmpdir
        return True
    except Exception:
        return False


def kernel(x, w1, w_rec, w2):
    from concourse.bass_utils import run_bass_kernel_spmd

    nc = build_nc()
    in_maps = prep_inputs(x, w1, w_rec, w2)
    trace = os.environ.get("SNN_TRACE", "0") == "1" and _setup_trace()
    res = run_bass_kernel_spmd(nc, in_maps, list(range(NCORES)), trace=trace)
    _LAST["exec_time_ns"] = res.exec_time_ns
    _LAST["results"] = res
    out = np.empty((B, O), dtype=np.float32)
    for c in range(NCORES):
        out[c * BL:(c + 1) * BL, :] = res.results[c]["out"].T
    return out

